# revision 1
# baseline (speedup 1.0000x reference)
"""DynamicConv (MoE-routed 1x1 conv) Trainium2 kernel.

Data-parallel over batch: 8 cores x 4 samples. Each core:
  - routing MLP (3-layer, exact GELU) + softmax on its 4 samples
  - mixes the K=8 expert kernels per sample (DVE AXPY chain)
  - per-sample 256x256 @ 256x4096 matmul on TensorE (float32r single-pass)

Main matmuls run with float32r operands: fp32 bits streamed through the PE
in one pass (4x the fp32 matmul rate) at ~1.5e-4 relative error (vs 2.3e-3
for bf16). The kernel is then HBM-bound: ~36 MB/core at ~358 GB/s.

Problem constants are hardcoded (self-contained; no sibling imports):
  x [32, 256, 4096] f32, embedding [32, 128] f32,
  W0 [128,128], b0 [128], W1 [128,128], b1 [128], W2 [128,8], b2 [8],
  weight [8, 256, 256, 1], bias_k [8, 256]  -> out [32, 256, 4096] f32
"""

import numpy as np

import concourse.bacc as bacc
import concourse.mybir as mybir
import concourse.tile as tile
from concourse import bass_utils

F32 = mybir.dt.float32
F32R = mybir.dt.float32r
AF = mybir.ActivationFunctionType
ALU = mybir.AluOpType

MM_DT = F32R  # set to F32 for exact-fp32 main matmuls (4x slower PE)

N_CORES = 8
BS = 32
BPC = BS // N_CORES  # samples per core
IN_C = 256
OUT_C = 256
H = 4096
K = 8
D_EMBD = 128
HID = 128
N_IT = IN_C // 128   # input-channel tiles
N_OT = OUT_C // 128  # output-channel tiles
HC = 512             # h-chunk (one PSUM bank of fp32)
N_HC = H // HC

# params blob column layout [128 partitions x P_COLS]
C_W0 = 0
C_W1 = C_W0 + HID          # 128
C_W2 = C_W1 + HID          # 256
C_B0 = C_W2 + K            # 264
C_B1 = C_B0 + 1            # 265
C_EMBT = C_B1 + 1          # 266
C_B2 = C_EMBT + BPC        # 270  (partitions 0:K)
C_BK = C_B2 + 1            # 271  (partitions 0:K)
C_ID8 = C_BK + OUT_C       # 527  (partitions 0:K)
P_COLS = C_ID8 + K         # 535

_PROG = None  # compiled program cache


def _build_program(repeat=1):
    nc = bacc.Bacc("TRN2", target_bir_lowering=False, debug=False)

    xs = nc.dram_tensor("xs", [BPC, IN_C, H], MM_DT, kind="ExternalInput").ap()
    # wta[il, k, it, o] = weight[k, o, it*128+il]
    wta = nc.dram_tensor("wta", [128, K * N_IT * OUT_C], F32,
                         kind="ExternalInput").ap()
    params = nc.dram_tensor("params", [128, P_COLS], F32,
                            kind="ExternalInput").ap()
    out = nc.dram_tensor("out", [BPC, OUT_C, H], F32, kind="ExternalOutput").ap()

    with tile.TileContext(nc) as tc:
        with (
            tc.tile_pool(name="consts", bufs=1) as cpool,
            tc.tile_pool(name="rsb", bufs=1) as rsb,
            tc.tile_pool(name="rps", bufs=1, space="PSUM") as rps,
            tc.tile_pool(name="mix", bufs=4) as mixp,
            tc.tile_pool(name="xin", bufs=2) as xinp,
            tc.tile_pool(name="osb", bufs=4) as osbp,
            tc.tile_pool(name="mps", bufs=7, space="PSUM") as mps,
        ):
            # ---- constant loads (2 big DMAs, SWDGE so they overlap the
            # x/out HWDGE streams on their own queue) ----
            pa = cpool.tile([128, P_COLS], F32, tag="params")
            nc.gpsimd.dma_start(pa[:], params[:])
            w0_sb = pa[:, C_W0:C_W0 + HID]
            w1_sb = pa[:, C_W1:C_W1 + HID]
            w2_sb = pa[:, C_W2:C_W2 + K]
            b0_sb = pa[:, C_B0:C_B0 + 1]
            b1_sb = pa[:, C_B1:C_B1 + 1]
            embT_sb = pa[:, C_EMBT:C_EMBT + BPC]
            b2_sb = pa[0:K, C_B2:C_B2 + 1]
            bk_sb = pa[0:K, C_BK:C_BK + OUT_C]
            id8_sb = pa[0:K, C_ID8:C_ID8 + K]

            wtall = cpool.tile([128, K * N_IT * OUT_C], F32, tag="wtall")
            nc.gpsimd.dma_start(wtall[:], wta[:])

            def wt_sb(k, it):
                off = (k * N_IT + it) * OUT_C
                return wtall[:, off:off + OUT_C]

            ones_sb = cpool.tile([1, 128], F32, tag="ones")
            nc.vector.memset(ones_sb[:], 1.0)

            for rep in range(repeat):
                # ---- routing MLP (transposed orientation, all 4 samples) ----
                p1 = rps.tile([HID, BPC], F32, tag="rp")
                nc.tensor.matmul(p1[:], w0_sb, embT_sb, start=True, stop=True)
                h1_sb = rsb.tile([HID, BPC], F32, tag="h1")
                nc.scalar.activation(h1_sb[:], p1[:], AF.Gelu, bias=b0_sb)

                p2 = rps.tile([HID, BPC], F32, tag="rp")
                nc.tensor.matmul(p2[:], w1_sb, h1_sb[:], start=True, stop=True)
                h2_sb = rsb.tile([HID, BPC], F32, tag="h2")
                nc.scalar.activation(h2_sb[:], p2[:], AF.Gelu, bias=b1_sb)

                p3 = rps.tile([K, BPC], F32, tag="rp")
                nc.tensor.matmul(p3[:], w2_sb, h2_sb[:], start=True, stop=True)
                lT_sb = rsb.tile([K, BPC], F32, tag="lT")
                nc.scalar.activation(lT_sb[:], p3[:], AF.Identity, bias=b2_sb)

                # logitsT [K, BPC] -> logits [BPC, K]; softmax over free dim.
                # Logits are O(1) here so exp without max-subtraction is safe.
                p4 = rps.tile([BPC, K], F32, tag="rp")
                nc.tensor.transpose(p4[:], lT_sb[:], id8_sb)
                e_sb = rsb.tile([BPC, K], F32, tag="e")
                nc.scalar.activation(e_sb[:], p4[:], AF.Exp)
                s_sb = rsb.tile([BPC, 1], F32, tag="s")
                nc.vector.reduce_sum(s_sb[:], e_sb[:], axis=mybir.AxisListType.X)
                r_sb = rsb.tile([BPC, 1], F32, tag="r")
                nc.vector.reciprocal(r_sb[:], s_sb[:])
                att_sb = rsb.tile([BPC, K], F32, tag="att")
                nc.vector.tensor_scalar_mul(att_sb[:], e_sb[:], r_sb[:, 0:1])

                # attT [K, BPC] for the bias mix
                p5 = rps.tile([K, BPC], F32, tag="rp")
                nc.tensor.transpose(p5[:], att_sb[:], id8_sb[0:BPC, 0:BPC])
                attT_sb = rsb.tile([K, BPC], F32, tag="attT")
                nc.vector.tensor_copy(attT_sb[:], p5[:])

                # agg_bT[ot] [128, BPC] = bias_k[:, ot].T @ att.T
                aggb_sb = []
                for ot in range(N_OT):
                    p6 = rps.tile([128, BPC], F32, tag="rp")
                    nc.tensor.matmul(p6[:], bk_sb[:, ot * 128:(ot + 1) * 128],
                                     attT_sb[:], start=True, stop=True)
                    a = rsb.tile([128, BPC], F32, tag=f"aggb{ot}", name=f"aggb{ot}")
                    nc.vector.tensor_copy(a[:], p6[:])
                    aggb_sb.append(a)

                # broadcast att to all 128 partitions: attB [128, BPC*K]
                att_flat = rsb.tile([1, BPC * K], F32, tag="attf")
                nc.gpsimd.dma_start(att_flat[:], att_sb[:])
                p7 = rps.tile([128, BPC * K], F32, tag="rp")
                nc.tensor.matmul(p7[:], ones_sb[:], att_flat[:], start=True, stop=True)
                attB_sb = rsb.tile([128, BPC * K], F32, tag="attB")
                nc.vector.tensor_copy(attB_sb[:], p7[:])

                # ---- mix expert kernels + main per-sample matmul ----
                for b in range(BPC):
                    mixT = []
                    for it in range(N_IT):
                        m = mixp.tile([128, OUT_C], F32, tag=f"mix{it}",
                                      name=f"mix_b{b}_{it}")
                        a0 = attB_sb[:, b * K:b * K + 1]
                        nc.vector.tensor_scalar_mul(m[:], wt_sb(0, it), a0)
                        for k in range(1, K - 1):
                            ak = attB_sb[:, b * K + k:b * K + k + 1]
                            nc.vector.scalar_tensor_tensor(
                                m[:], wt_sb(k, it), ak, m[:],
                                op0=ALU.mult, op1=ALU.add)
                        # last AXPY rounds the accumulator into f32r
                        mr = mixp.tile([128, OUT_C], MM_DT, tag=f"mixr{it}",
                                       name=f"mixr_b{b}_{it}")
                        ak = attB_sb[:, b * K + (K - 1):b * K + K]
                        nc.vector.scalar_tensor_tensor(
                            mr[:], wt_sb(K - 1, it), ak, m[:],
                            op0=ALU.mult, op1=ALU.add)
                        mixT.append(mr)

                    # two 2 MB DMAs per sample (always on SP's ring: reads
                    # depend only on slot release, never behind compute).
                    # Separate tiles let the it=0 matmuls start while the
                    # it=1 half is still in flight — shortens the tail.
                    x_t = []
                    for it in range(N_IT):
                        xh = xinp.tile([128, H], MM_DT, tag=f"x{it}",
                                       name=f"x_b{b}_{it}")
                        nc.sync.dma_start(xh[:],
                                          xs[b, it * 128:(it + 1) * 128, :])
                        x_t.append(xh)

                    for ot in range(N_OT):
                        o_sb = osbp.tile([128, H], F32, tag="o",
                                         name=f"o_b{b}_{ot}")
                        for hc in range(N_HC):
                            ps = mps.tile([128, HC], F32, tag="mm")
                            for it in range(N_IT):
                                nc.tensor.matmul(
                                    ps[:],
                                    mixT[it][:, ot * 128:(ot + 1) * 128],
                                    x_t[it][:, hc * HC:(hc + 1) * HC],
                                    start=(it == 0), stop=(it == N_IT - 1))
                            dst = o_sb[:, hc * HC:(hc + 1) * HC]
                            bias_ap = aggb_sb[ot][:, b:b + 1]
                            if hc % 2 == 0:
                                nc.scalar.activation(dst, ps[:], AF.Identity,
                                                     bias=bias_ap)
                            else:
                                nc.vector.tensor_scalar(dst, ps[:], bias_ap, None,
                                                        op0=ALU.add)
                        # each output tile leaves as two 1 MB halves on the
                        # two DMA paths that don't carry the x reads (ACT
                        # HWDGE + GpSimd SWDGE): writes overlap reads, SP's
                        # read stream stays uncontended, tail transfer halves
                        orows = out[b, ot * 128:(ot + 1) * 128, :]
                        nc.gpsimd.dma_start(orows[:, 0:H // 2],
                                            o_sb[:, 0:H // 2])
                        nc.scalar.dma_start(orows[:, H // 2:H],
                                            o_sb[:, H // 2:H])

    nc.compile()
    return nc


def _get_program():
    global _PROG
    if _PROG is None:
        _PROG = _build_program()
    return _PROG


def build_in_maps(inputs):
    x = np.ascontiguousarray(np.asarray(inputs["x"], dtype=np.float32))
    emb = np.asarray(inputs["embedding"], dtype=np.float32)
    W0 = np.asarray(inputs["W0"], dtype=np.float32)
    b0 = np.asarray(inputs["b0"], dtype=np.float32)
    W1 = np.asarray(inputs["W1"], dtype=np.float32)
    b1 = np.asarray(inputs["b1"], dtype=np.float32)
    W2 = np.asarray(inputs["W2"], dtype=np.float32)
    b2 = np.asarray(inputs["b2"], dtype=np.float32)
    weight = np.asarray(inputs["weight"], dtype=np.float32)[..., 0]  # [K, O, I]
    bias_k = np.asarray(inputs["bias_k"], dtype=np.float32)

    # wta[il, (k, it, o)] = weight[k, o, it*128+il]
    wta = np.ascontiguousarray(
        weight.transpose(2, 0, 1)           # [I, K, O]
        .reshape(N_IT, 128, K, OUT_C)       # [it, il, K, O]
        .transpose(1, 2, 0, 3)              # [il, K, it, O]
        .reshape(128, K * N_IT * OUT_C))

    base = np.zeros((128, P_COLS), dtype=np.float32)
    base[:, C_W0:C_W0 + HID] = W0
    base[:, C_W1:C_W1 + HID] = W1
    base[:, C_W2:C_W2 + K] = W2
    base[:, C_B0] = b0
    base[:, C_B1] = b1
    base[0:K, C_B2] = b2
    base[0:K, C_BK:C_BK + OUT_C] = bias_k
    base[0:K, C_ID8:C_ID8 + K] = np.eye(K, dtype=np.float32)

    in_maps = []
    for c in range(N_CORES):
        sl = slice(c * BPC, (c + 1) * BPC)
        p = base.copy()
        p[:, C_EMBT:C_EMBT + BPC] = emb[sl].T
        in_maps.append({
            "xs": np.ascontiguousarray(x[sl]),
            "wta": wta,
            "params": p,
        })
    return in_maps


def run(inputs, trace=False):
    nc = _get_program()
    in_maps = build_in_maps(inputs)
    res = bass_utils.run_bass_kernel_spmd(
        nc, in_maps, core_ids=list(range(N_CORES)), trace=trace)
    out = np.concatenate([res.results[c]["out"] for c in range(N_CORES)], axis=0)
    return out, res


def kernel(**inputs):
    out, _ = run(inputs, trace=False)
    return out



# revision 3
# speedup vs baseline: 1.1158x; 1.1158x over previous
"""DynamicConv (MoE-routed 1x1 conv) Trainium2 kernel, v2.

Data-parallel over batch: 8 cores x 4 samples. Per core:
  - routing MLP (3-layer, exact GELU) in transposed orientation; softmax
    numerator/denominator split: unnormalized exps drive the kernel mix,
    1/sum is folded into the PSUM eviction (scale+bias in one ACT/DVE op).
  - logits reach all 128 partitions with no SBUF->SBUF DMA: W2*h2
    broadcast-AP product + ones-matmul collapse + gpsimd partition_broadcast.
  - expert kernels mixed per sample with AXPY chains split across Pool
    (it0 half) and DVE (it1 half); f32 accumulate, f16 result.
  - main matmuls on PE in f16 (1 cycle/row), f32 PSUM, 10 warmup fillers
    keep the PE pstate ramped before the main stream.
  - evictions (PSUM->SBUF f16) and out-writes phase-balanced across
    ACT/DVE and Pool/ACT/SP DMA queues; all HBM streams in f16.

Hardcoded problem: x[32,256,4096] f32, embedding[32,128] f32,
W0[128,128] b0[128] W1[128,128] b1[128] W2[128,8] b2[8],
weight[8,256,256,1], bias_k[8,256] -> out[32,256,4096] f32.
"""

import numpy as np

import concourse.bacc as bacc
import concourse.mybir as mybir
import concourse.tile as tile
from concourse import bass_utils

F32 = mybir.dt.float32
F16 = mybir.dt.float16
AF = mybir.ActivationFunctionType
ALU = mybir.AluOpType

N_CORES = 8
BS = 32
BPC = BS // N_CORES
IN_C = 256
OUT_C = 256
H = 4096
K = 8
D_EMBD = 128
HID = 128
N_IT = IN_C // 128
N_OT = OUT_C // 128

# params tile 1a/1b (arrive first; routing MLP front). exp(b2) is folded
# into wta and bias_k on the host, so the row path broadcasts raw exp(l)
# and the softmax denominator comes from an expb2-weighted 8x4 matmul.
P1_W0 = 0                 # [128, 128] W0[d, h]
P1_EMBT = 128             # [128, 4]  emb.T
P1_B0 = 132               # [128, 1]
P1A_COLS = 133
P1B_W1 = 0                # [128, 128] W1
P1B_B1 = 128              # [128, 1]
P1B_ONES = 129            # [128, 1] all ones
P1B_COLS = 130

# params tile 2
P2_W2 = 0                 # [128, 8] W2[h, k]
P2_EB2C = 8               # rows 0:8, col: exp(b2[k])
P2_BK = 9                 # rows 0:8, [8, 256] exp(b2[k])*bias_k
P2_COLS = 265

# evict engine by global chunk order index g: DVE (the dedicated evictor)
# takes odd g plus a few evens; ACT the rest. The final chunk (g=31) is
# split into two 512-col halves done by ACT and DVE concurrently.
# explicit evictor assignment by global chunk order index (g31 is split)
DVE_EV = {25, 27, 29}


def ev_on_dve(g):
    return g in DVE_EV

_PROG = None


def _build_program():
    nc = bacc.Bacc("TRN2", target_bir_lowering=False, debug=False)

    xs = nc.dram_tensor("xs", [BPC, IN_C, H], F16, kind="ExternalInput").ap()
    # wta[il, it*2048 + k*256 + o] = weight[k, o, it*128+il]
    wta = nc.dram_tensor("wta", [128, N_IT * K * OUT_C], F16,
                         kind="ExternalInput").ap()
    params1a = nc.dram_tensor("params1a", [128, P1A_COLS], F32,
                              kind="ExternalInput").ap()
    params1b = nc.dram_tensor("params1b", [128, P1B_COLS], F32,
                              kind="ExternalInput").ap()
    params2 = nc.dram_tensor("params2", [128, P2_COLS], F32,
                             kind="ExternalInput").ap()
    out = nc.dram_tensor("out", [BPC, OUT_C, H], F16, kind="ExternalOutput").ap()

    with tile.TileContext(nc) as tc:
        with (
            tc.tile_pool(name="consts", bufs=1) as cpool,
            tc.tile_pool(name="mix32", bufs=1) as mx32,
            tc.tile_pool(name="mix16", bufs=4) as mx16,
            tc.tile_pool(name="xin", bufs=4) as xinp,
            tc.tile_pool(name="osb", bufs=4) as osbp,
            tc.tile_pool(name="o3sb", bufs=4) as o3sbp,
            tc.tile_pool(name="mps", bufs=1, space="PSUM") as mps,
        ):
            # ---- param + weight loads: pa1 + it0-wta on Pool, pa2 +
            # it1-wta on SP (then SP streams x). Keeps every queue's
            # first-needed bytes at its front.
            pa1 = cpool.tile([128, P1A_COLS], F32, tag="pa1")
            nc.gpsimd.dma_start(pa1[:], params1a[:])
            pa1b = cpool.tile([128, P1B_COLS], F32, tag="pa1b")
            nc.gpsimd.dma_start(pa1b[:], params1b[:])
            pa2 = cpool.tile([128, P2_COLS], F32, tag="pa2")
            nc.gpsimd.dma_start(pa2[:], params2[:])

            wt_q = []
            for q in range(4):
                t = cpool.tile([128, 1024], F16, tag=f"wtq{q}", name=f"wtq{q}")
                eng = nc.sync if q < 2 else nc.gpsimd
                eng.dma_start(t[:], wta[:, q * 1024:(q + 1) * 1024])
                wt_q.append(t)

            # ---- ACT Gelu table warmup (Exp loads once, before eT/erow;
            # keep every Gelu use before the first Exp use) ----
            warm = cpool.tile([128, 1], F32, tag="warm")
            nc.vector.memset(warm[:], 0.25)
            warm_o = cpool.tile([128, 1], F32, tag="warm_o")
            nc.scalar.activation(warm_o[:], warm[:], AF.Gelu)

            def wt_sb(k, it):
                q = it * 2 + k // 4
                off = (k % 4) * 256
                return wt_q[q][:, off:off + 256]

            # ---- routing MLP (transposed; all samples at once) ----
            # routing PSUMs live in slices of one pm0 tile; the WAW chain
            # through it serializes them naturally and frees no extra bank
            rpt = mps.tile([128, 1024], F32, tag="pm0", name="rpt")
            p1 = rpt[:, 0:BPC]
            nc.tensor.matmul(p1, pa1[:, P1_W0:P1_W0 + HID],
                             pa1[:, P1_EMBT:P1_EMBT + BPC], start=True, stop=True)
            h1 = cpool.tile([HID, BPC], F32, tag="h1")
            nc.scalar.activation(h1[:], p1, AF.Gelu,
                                 bias=pa1[:, P1_B0:P1_B0 + 1])

            p2 = rpt[:, 8:8 + BPC]
            nc.tensor.matmul(p2, pa1b[:, P1B_W1:P1B_W1 + HID], h1[:],
                             start=True, stop=True)
            h2 = cpool.tile([HID, BPC], F32, tag="h2")
            nc.scalar.activation(h2[:], p2, AF.Gelu,
                                 bias=pa1b[:, P1B_B1:P1B_B1 + 1])

            # column path: eT[k, b] = exp(l[k, b] + b2[k])  (for agg bias)
            p3 = rpt[0:K, 16:16 + BPC]
            nc.tensor.matmul(p3, pa2[:, P2_W2:P2_W2 + K], h2[:],
                             start=True, stop=True)
            # exp(x) = (1 + tanh(x/2)) / (1 - tanh(x/2)): tanh lives in the
            # same act table as exact Gelu, so no second table load.
            eT = cpool.tile([K, BPC], F32, tag="eT")
            tcol = cpool.tile([K, BPC], F32, tag="tcol")
            nc.scalar.activation(tcol[:], p3, AF.Tanh, scale=0.5)
            dencol = cpool.tile([K, BPC], F32, tag="dencol")
            nc.vector.tensor_scalar(dencol[:], tcol[:], -1.0, 1.0,
                                    op0=ALU.mult, op1=ALU.add)
            nc.vector.reciprocal(dencol[:], dencol[:])
            numcol = cpool.tile([K, BPC], F32, tag="numcol")
            nc.vector.tensor_scalar(numcol[:], tcol[:], 1.0, None, op0=ALU.add)
            nc.vector.tensor_tensor(eT[:], numcol[:], dencol[:], op=ALU.mult)

            # row path: l_row[0, (b,k)] = sum_h W2[h,k] * h2[h,b]
            prod = cpool.tile([128, BPC * K], F32, tag="prod")
            w2b = pa2[:, P2_W2:P2_W2 + K].unsqueeze(1).broadcast_to([128, BPC, K])
            h2b = h2[:].unsqueeze(2).broadcast_to([128, BPC, K])
            nc.vector.tensor_tensor(
                prod[:].rearrange("p (b k) -> p b k", b=BPC), w2b, h2b,
                op=ALU.mult)
            lrow = rpt[0:1, 32:32 + BPC * K]
            nc.tensor.matmul(lrow, pa1b[:, P1B_ONES:P1B_ONES + 1], prod[:],
                             start=True, stop=True)
            erow = cpool.tile([1, BPC * K], F32, tag="erow")
            trow = cpool.tile([1, BPC * K], F32, tag="trow")
            nc.scalar.activation(trow[:], lrow, AF.Tanh, scale=0.5)
            denrow = cpool.tile([1, BPC * K], F32, tag="denrow")
            nc.vector.tensor_scalar(denrow[:], trow[:], -1.0, 1.0,
                                    op0=ALU.mult, op1=ALU.add)
            nc.vector.reciprocal(denrow[:], denrow[:])
            numrow = cpool.tile([1, BPC * K], F32, tag="numrow")
            nc.vector.tensor_scalar(numrow[:], trow[:], 1.0, None, op0=ALU.add)
            nc.vector.tensor_tensor(erow[:], numrow[:], denrow[:], op=ALU.mult)

            # broadcast raw exps immediately: the mix needs only these
            eB = cpool.tile([128, BPC * K], F32, tag="eB")
            nc.gpsimd.partition_broadcast(eB[:], erow[:])

            # softmax denominator: s[1, b] = sum_k exp(b2[k]) * eT[k, b]
            srow = rpt[0:1, 64:64 + BPC]
            nc.tensor.matmul(srow, pa2[0:K, P2_EB2C:P2_EB2C + 1], eT[:],
                             start=True, stop=True)
            rrow = cpool.tile([1, BPC], F32, tag="rrow")
            nc.vector.reciprocal(rrow[:], srow)
            rBt = cpool.tile([128, BPC], F32, tag="rBt")
            nc.gpsimd.partition_broadcast(rBt[:], rrow[:])

            def sc(b, k):
                return eB[:, b * K + k:b * K + k + 1]

            # agg bias column-path matmuls
            pag0 = rpt[:, 128:128 + BPC]
            nc.tensor.matmul(pag0, pa2[0:K, P2_BK:P2_BK + 128], eT[:],
                             start=True, stop=True)
            pag1 = rpt[:, 256:256 + BPC]
            nc.tensor.matmul(pag1, pa2[0:K, P2_BK + 128:P2_BK + 256], eT[:],
                             start=True, stop=True)

            aggb = cpool.tile([128, N_OT * BPC], F32, tag="aggb")

            # ---- mix chains ----
            # pre-allocate result tiles in sample order (bufs=4: no recycling)
            mh = {}
            for b in range(BPC):
                for it in range(N_IT):
                    mh[(b, it)] = mx16.tile([128, 256], F16, tag=f"m16_{it}",
                                            name=f"m16_{b}_{it}")

            def mix_chain(eng, b, it, lo, hi):
                m32 = mx32.tile([128, 256], F32, tag=f"m32_{it}",
                                name=f"m32_{b}_{it}_{lo}")
                m16 = mh[(b, it)]
                eng.tensor_scalar_mul(m32[:, lo:hi], wt_sb(0, it)[:, lo:hi],
                                      sc(b, 0))
                for k in range(1, K - 1):
                    eng.scalar_tensor_tensor(
                        m32[:, lo:hi], wt_sb(k, it)[:, lo:hi], sc(b, k),
                        m32[:, lo:hi], op0=ALU.mult, op1=ALU.add)
                eng.scalar_tensor_tensor(
                    m16[:, lo:hi], wt_sb(K - 1, it)[:, lo:hi], sc(b, K - 1),
                    m32[:, lo:hi], op0=ALU.mult, op1=ALU.add)

            # All mix chains on DVE (GPSIMD cannot run TensorScalarPtr on
            # real TRN2); b0's chains are ot-split for an earlier PE start.
            mix_chain(nc.vector, 0, 0, 0, 128)
            mix_chain(nc.vector, 0, 1, 0, 128)
            mix_chain(nc.vector, 0, 0, 128, 256)
            mix_chain(nc.vector, 0, 1, 128, 256)
            mix_chain(nc.vector, 1, 0, 0, 256)
            mix_chain(nc.vector, 1, 1, 0, 256)
            mix_chain(nc.vector, 2, 0, 0, 256)
            mix_chain(nc.vector, 2, 1, 0, 256)
            mix_chain(nc.vector, 3, 0, 0, 256)
            mix_chain(nc.vector, 3, 1, 0, 256)
            nc.vector.tensor_tensor(aggb[:, 0:BPC], pag0, rBt[:],
                                    op=ALU.mult)
            nc.vector.tensor_tensor(aggb[:, BPC:2 * BPC], pag1, rBt[:],
                                    op=ALU.mult)

            # ---- x loads (SP): per sample, per it, two 2048-col halves ----
            x_t = {}
            for b in range(BPC):
                for half in range(2):
                    for it in range(N_IT):
                        t = xinp.tile([128, 2048], F16, tag=f"x{it}{half}",
                                      name=f"x_{b}_{it}_{half}")
                        x_t[(b, it, half)] = t
            for b in range(BPC):
                if b == 0:
                    # interleaved 1024-col quarters: x arrives in the order
                    # the first sample's psum chunks consume it
                    for qq in range(4):
                        for it in range(N_IT):
                            t = x_t[(b, it, qq // 2)]
                            lo = (qq % 2) * 1024
                            nc.sync.dma_start(
                                t[:, lo:lo + 1024],
                                xs[b, it * 128:(it + 1) * 128,
                                   qq * 1024:(qq + 1) * 1024])
                else:
                    for half in range(2):
                        for it in range(N_IT):
                            t = x_t[(b, it, half)]
                            nc.sync.dma_start(
                                t[:], xs[b, it * 128:(it + 1) * 128,
                                         half * 2048:(half + 1) * 2048])

            osb = {}
            for b in range(BPC):
                for ot in range(N_OT):
                    if (b, ot) == (3, 1):
                        for q in range(3):
                            osb[(b, ot, q)] = o3sbp.tile(
                                [128, 1024], F16, tag="o3",
                                name=f"o3_{b}_{ot}_{q}")
                        for sq in range(2):
                            osb[(b, ot, 3, sq)] = o3sbp.tile(
                                [128, 512], F16, tag="o3s",
                                name=f"o3s_{b}_{ot}_{sq}")
                    else:
                        for hh in range(2):
                            osb[(b, ot, hh)] = osbp.tile(
                                [128, 2048], F16, tag="o",
                                name=f"o_{b}_{ot}_{hh}")

            def emit_chunk(g, b, ot, c):
                """Matmuls + eviction for one 1024-col psum chunk."""
                ps = mps.tile([128, 1024], F32, tag=f"pm{g % 4}",
                              name=f"ps_{b}_{ot}_{c}")
                half = c // 2
                for s in range(2):
                    lo = (c % 2) * 1024 + s * 512
                    for it in range(N_IT):
                        nc.tensor.matmul(
                            ps[:, s * 512:(s + 1) * 512],
                            mh[(b, it)][:, ot * 128:(ot + 1) * 128],
                            x_t[(b, it, half)][:, lo:lo + 512],
                            start=(it == 0), stop=(it == N_IT - 1))
                bias_ap = aggb[:, ot * BPC + b:ot * BPC + b + 1]
                scale_ap = rBt[:, b:b + 1]
                if g == 31:
                    # final chunk: two 512-col halves, separate out tiles,
                    # evicted concurrently on ACT and DVE
                    nc.scalar.activation(osb[(b, ot, 3, 0)][:],
                                         ps[:, 0:512], AF.Identity,
                                         bias=bias_ap, scale=scale_ap)
                    nc.vector.tensor_scalar(osb[(b, ot, 3, 1)][:],
                                            ps[:, 512:1024],
                                            scale_ap, bias_ap,
                                            op0=ALU.mult, op1=ALU.add)
                    return
                if (b, ot) == (3, 1):
                    otile, olo = osb[(b, ot, c)], 0
                else:
                    otile, olo = osb[(b, ot, c // 2)], (c % 2) * 1024
                dst = otile[:, olo:olo + 1024]
                if ev_on_dve(g):
                    nc.vector.tensor_scalar(dst, ps[:], scale_ap, bias_ap,
                                            op0=ALU.mult, op1=ALU.add)
                else:
                    nc.scalar.activation(dst, ps[:], AF.Identity,
                                         bias=bias_ap, scale=scale_ap)

            def emit_writes(b, ot):
                # ACT: early halves (slack between its evicts); Pool: mid
                # halves deferred past its mix chains; SP: all of b3 (its x
                # stream is done by then).
                if (b, ot) == (3, 1):
                    for q in range(3):
                        orows = out[b, ot * 128:(ot + 1) * 128,
                                    q * 1024:(q + 1) * 1024]
                        nc.sync.dma_start(orows, osb[(b, ot, q)][:])
                    for sq in range(2):
                        orows = out[b, ot * 128:(ot + 1) * 128,
                                    3072 + sq * 512:3072 + (sq + 1) * 512]
                        nc.sync.dma_start(orows, osb[(b, ot, 3, sq)][:])
                    return
                for hh in range(2):
                    orows = out[b, ot * 128:(ot + 1) * 128,
                                hh * 2048:(hh + 1) * 2048]
                    src = osb[(b, ot, hh)][:]
                    nc.gpsimd.dma_start(orows, src)

            # chunk order per sample: h0 chunks of both ots first, then h1
            # (x's second half is needed 3.4us into the sample, giving the
            # serial SP x-stream slack to stay ahead of PE)
            g = 0
            B0_ORDER = ((0, 0), (1, 0), (0, 1), (1, 1),
                        (0, 2), (1, 2), (0, 3), (1, 3))
            ORDER = ((0, 0), (0, 1), (1, 0), (1, 1),
                     (0, 2), (0, 3), (1, 2), (1, 3))
            for b in range(BPC):
                for ot, c in (B0_ORDER if b == 0 else ORDER):
                    emit_chunk(g, b, ot, c)
                    g += 1
                emit_writes(b, 0)
                emit_writes(b, 1)

    nc.compile()
    return nc


def _get_program():
    global _PROG
    if _PROG is None:
        _PROG = _build_program()
    return _PROG


def build_in_maps(inputs):
    x = np.asarray(inputs["x"], dtype=np.float32)
    emb = np.asarray(inputs["embedding"], dtype=np.float32)
    W0 = np.asarray(inputs["W0"], dtype=np.float32)
    b0 = np.asarray(inputs["b0"], dtype=np.float32)
    W1 = np.asarray(inputs["W1"], dtype=np.float32)
    b1 = np.asarray(inputs["b1"], dtype=np.float32)
    W2 = np.asarray(inputs["W2"], dtype=np.float32)
    b2 = np.asarray(inputs["b2"], dtype=np.float32)
    weight = np.asarray(inputs["weight"], dtype=np.float32)[..., 0]  # [K,O,I]
    bias_k = np.asarray(inputs["bias_k"], dtype=np.float32)

    x16 = x.astype(np.float16)

    expb2 = np.exp(b2.astype(np.float64))
    wscaled = weight * expb2[:, None, None].astype(np.float32)
    wta = np.ascontiguousarray(
        wscaled.transpose(2, 0, 1)            # [I, K, O]
        .reshape(N_IT, 128, K, OUT_C)         # [it, il, K, O]
        .transpose(1, 0, 2, 3)                # [il, it, K, O]
        .reshape(128, N_IT * K * OUT_C)).astype(np.float16)

    pa1 = np.zeros((128, P1A_COLS), dtype=np.float32)
    pa1[:, P1_W0:P1_W0 + HID] = W0
    pa1[:, P1_B0] = b0
    pa1b = np.zeros((128, P1B_COLS), dtype=np.float32)
    pa1b[:, P1B_W1:P1B_W1 + HID] = W1
    pa1b[:, P1B_B1] = b1
    pa1b[:, P1B_ONES] = 1.0

    pa2 = np.zeros((128, P2_COLS), dtype=np.float32)
    pa2[:, P2_W2:P2_W2 + K] = W2
    pa2[0:K, P2_EB2C] = expb2.astype(np.float32)
    pa2[0:K, P2_BK:P2_BK + OUT_C] = (
        bias_k * expb2[:, None]).astype(np.float32)

    in_maps = []
    for c in range(N_CORES):
        sl = slice(c * BPC, (c + 1) * BPC)
        p1 = pa1.copy()
        p1[:, P1_EMBT:P1_EMBT + BPC] = emb[sl].T
        in_maps.append({
            "xs": np.ascontiguousarray(x16[sl]),
            "wta": wta,
            "params1a": p1,
            "params1b": pa1b,
            "params2": pa2,
        })
    return in_maps


def run(inputs, trace=False):
    nc = _get_program()
    in_maps = build_in_maps(inputs)
    res = bass_utils.run_bass_kernel_spmd(
        nc, in_maps, core_ids=list(range(N_CORES)), trace=trace)
    out = np.concatenate(
        [res.results[c]["out"] for c in range(N_CORES)], axis=0
    ).astype(np.float32)
    return out, res


def kernel(**inputs):
    out, _ = run(inputs, trace=False)
    return out


# revision 4
# speedup vs baseline: 1.6099x; 1.4428x over previous
"""DynamicConv (MoE-routed 1x1 conv) Trainium2 kernel, v2.

Data-parallel over batch: 8 cores x 4 samples. Per core:
  - routing MLP (3-layer, exact GELU) in transposed orientation; softmax
    numerator/denominator split: unnormalized exps drive the kernel mix,
    1/sum is folded into the PSUM eviction (scale+bias in one ACT/DVE op).
  - logits reach all 128 partitions with no SBUF->SBUF DMA: W2*h2
    broadcast-AP product + ones-matmul collapse + gpsimd partition_broadcast.
  - expert kernels mixed per sample with AXPY chains split across Pool
    (it0 half) and DVE (it1 half); f32 accumulate, f16 result.
  - main matmuls on PE in f16 (1 cycle/row), f32 PSUM, 10 warmup fillers
    keep the PE pstate ramped before the main stream.
  - evictions (PSUM->SBUF f16) and out-writes phase-balanced across
    ACT/DVE and Pool/ACT/SP DMA queues; all HBM streams in f16.

Hardcoded problem: x[32,256,4096] f32, embedding[32,128] f32,
W0[128,128] b0[128] W1[128,128] b1[128] W2[128,8] b2[8],
weight[8,256,256,1], bias_k[8,256] -> out[32,256,4096] f32.
"""

import numpy as np

import concourse.bacc as bacc
import concourse.mybir as mybir
import concourse.tile as tile
from concourse import bass_utils

F32 = mybir.dt.float32
F16 = mybir.dt.float16
AF = mybir.ActivationFunctionType
ALU = mybir.AluOpType

N_CORES = 8
BS = 32
BPC = BS // N_CORES
IN_C = 256
OUT_C = 256
H = 4096
K = 8
D_EMBD = 128
HID = 128
N_IT = IN_C // 128
N_OT = OUT_C // 128

# params tile 1a/1b (arrive first; routing MLP front). exp(b2) is folded
# into wta and bias_k on the host, so the row path broadcasts raw exp(l)
# and the softmax denominator comes from an expb2-weighted 8x4 matmul.
P1_W0 = 0                 # [128, 128] W0[d, h]
P1_EMBT = 128             # [128, 4]  emb.T
P1_B0 = 132               # [128, 1]
P1A_COLS = 133
P1B_W1 = 0                # [128, 128] W1
P1B_B1 = 128              # [128, 1]
P1B_ONES = 129            # [128, 1] all ones
P1B_COLS = 130

# params tile 2
P2_W2 = 0                 # [128, 8] W2[h, k]
P2_EB2C = 8               # rows 0:8, col: exp(b2[k])
P2_BK = 9                 # rows 0:8, [8, 256] exp(b2[k])*bias_k
P2_COLS = 265

# evict engine by global chunk order index g: DVE (the dedicated evictor)
# takes odd g plus a few evens; ACT the rest. The final chunk (g=31) is
# split into two 512-col halves done by ACT and DVE concurrently.
# explicit evictor assignment by global chunk order index (g31 is split)
DVE_EV = {17, 19, 21, 23, 25, 27, 29}


def ev_on_dve(g):
    return g in DVE_EV

_PROG = None


def _build_program():
    nc = bacc.Bacc("TRN2", target_bir_lowering=False, debug=False)

    xs = nc.dram_tensor("xs", [BPC, IN_C, H], F16, kind="ExternalInput").ap()
    # wta[il, it*2048 + k*256 + o] = weight[k, o, it*128+il]
    wta = nc.dram_tensor("wta", [128, N_IT * K * OUT_C], F16,
                         kind="ExternalInput").ap()
    params1a = nc.dram_tensor("params1a", [128, P1A_COLS], F32,
                              kind="ExternalInput").ap()
    params1b = nc.dram_tensor("params1b", [128, P1B_COLS], F32,
                              kind="ExternalInput").ap()
    params2 = nc.dram_tensor("params2", [128, P2_COLS], F32,
                             kind="ExternalInput").ap()
    out = nc.dram_tensor("out", [BPC, OUT_C, H], F16, kind="ExternalOutput").ap()

    with tile.TileContext(nc) as tc:
        with (
            tc.tile_pool(name="consts", bufs=1) as cpool,
            tc.tile_pool(name="mix32", bufs=1) as mx32,
            tc.tile_pool(name="mix16", bufs=4) as mx16,
            tc.tile_pool(name="xin", bufs=4) as xinp,
            tc.tile_pool(name="osb", bufs=4) as osbp,
            tc.tile_pool(name="o3sb", bufs=4) as o3sbp,
            tc.tile_pool(name="mps", bufs=1, space="PSUM") as mps,
        ):
            # ---- param + weight loads: pa1 + it0-wta on Pool, pa2 +
            # it1-wta on SP (then SP streams x). Keeps every queue's
            # first-needed bytes at its front.
            pa1 = cpool.tile([128, P1A_COLS], F32, tag="pa1")
            nc.gpsimd.dma_start(pa1[:], params1a[:])
            pa1b = cpool.tile([128, P1B_COLS], F32, tag="pa1b")
            nc.gpsimd.dma_start(pa1b[:], params1b[:])
            pa2 = cpool.tile([128, P2_COLS], F32, tag="pa2")
            nc.gpsimd.dma_start(pa2[:], params2[:])

            wt_q = []
            for q in range(4):
                t = cpool.tile([128, 1024], F16, tag=f"wtq{q}", name=f"wtq{q}")
                eng = nc.sync if q < 2 else nc.gpsimd
                eng.dma_start(t[:], wta[:, q * 1024:(q + 1) * 1024])
                wt_q.append(t)

            # ---- ACT Gelu table warmup (Exp loads once, before eT/erow;
            # keep every Gelu use before the first Exp use) ----
            warm = cpool.tile([128, 1], F32, tag="warm")
            nc.vector.memset(warm[:], 0.25)
            warm_o = cpool.tile([128, 1], F32, tag="warm_o")
            nc.scalar.activation(warm_o[:], warm[:], AF.Gelu)

            def wt_sb(k, it):
                q = it * 2 + k // 4
                off = (k % 4) * 256
                return wt_q[q][:, off:off + 256]

            # ---- routing MLP (transposed; all samples at once) ----
            # routing PSUMs live in slices of one pm0 tile; the WAW chain
            # through it serializes them naturally and frees no extra bank
            rpt = mps.tile([128, 1024], F32, tag="pm0", name="rpt")
            p1 = rpt[:, 0:BPC]
            nc.tensor.matmul(p1, pa1[:, P1_W0:P1_W0 + HID],
                             pa1[:, P1_EMBT:P1_EMBT + BPC], start=True, stop=True)
            h1 = cpool.tile([HID, BPC], F32, tag="h1")
            nc.scalar.activation(h1[:], p1, AF.Gelu,
                                 bias=pa1[:, P1_B0:P1_B0 + 1])

            p2 = rpt[:, 8:8 + BPC]
            nc.tensor.matmul(p2, pa1b[:, P1B_W1:P1B_W1 + HID], h1[:],
                             start=True, stop=True)
            h2 = cpool.tile([HID, BPC], F32, tag="h2")
            nc.scalar.activation(h2[:], p2, AF.Gelu,
                                 bias=pa1b[:, P1B_B1:P1B_B1 + 1])

            # column path: eT[k, b] = exp(l[k, b] + b2[k])  (for agg bias)
            p3 = rpt[0:K, 16:16 + BPC]
            nc.tensor.matmul(p3, pa2[:, P2_W2:P2_W2 + K], h2[:],
                             start=True, stop=True)
            # exp(x) = (1 + tanh(x/2)) / (1 - tanh(x/2)): tanh lives in the
            # same act table as exact Gelu, so no second table load.
            eT = cpool.tile([K, BPC], F32, tag="eT")
            tcol = cpool.tile([K, BPC], F32, tag="tcol")
            nc.scalar.activation(tcol[:], p3, AF.Tanh, scale=0.5)
            dencol = cpool.tile([K, BPC], F32, tag="dencol")
            nc.vector.tensor_scalar(dencol[:], tcol[:], -1.0, 1.0,
                                    op0=ALU.mult, op1=ALU.add)
            nc.vector.reciprocal(dencol[:], dencol[:])
            numcol = cpool.tile([K, BPC], F32, tag="numcol")
            nc.vector.tensor_scalar(numcol[:], tcol[:], 1.0, None, op0=ALU.add)
            nc.vector.tensor_tensor(eT[:], numcol[:], dencol[:], op=ALU.mult)

            # row path: l_row[0, (b,k)] = sum_h W2[h,k] * h2[h,b]
            prod = cpool.tile([128, BPC * K], F32, tag="prod")
            w2b = pa2[:, P2_W2:P2_W2 + K].unsqueeze(1).broadcast_to([128, BPC, K])
            h2b = h2[:].unsqueeze(2).broadcast_to([128, BPC, K])
            nc.vector.tensor_tensor(
                prod[:].rearrange("p (b k) -> p b k", b=BPC), w2b, h2b,
                op=ALU.mult)
            lrow = rpt[0:1, 32:32 + BPC * K]
            nc.tensor.matmul(lrow, pa1b[:, P1B_ONES:P1B_ONES + 1], prod[:],
                             start=True, stop=True)
            erow = cpool.tile([1, BPC * K], F32, tag="erow")
            trow = cpool.tile([1, BPC * K], F32, tag="trow")
            nc.scalar.activation(trow[:], lrow, AF.Tanh, scale=0.5)
            denrow = cpool.tile([1, BPC * K], F32, tag="denrow")
            nc.vector.tensor_scalar(denrow[:], trow[:], -1.0, 1.0,
                                    op0=ALU.mult, op1=ALU.add)
            nc.vector.reciprocal(denrow[:], denrow[:])
            numrow = cpool.tile([1, BPC * K], F32, tag="numrow")
            nc.vector.tensor_scalar(numrow[:], trow[:], 1.0, None, op0=ALU.add)
            nc.vector.tensor_tensor(erow[:], numrow[:], denrow[:], op=ALU.mult)

            # broadcast raw exps immediately: the mix needs only these
            eB = cpool.tile([128, BPC * K], F32, tag="eB")
            nc.gpsimd.partition_broadcast(eB[:], erow[:])

            # softmax denominator: s[1, b] = sum_k exp(b2[k]) * eT[k, b]
            srow = rpt[0:1, 64:64 + BPC]
            nc.tensor.matmul(srow, pa2[0:K, P2_EB2C:P2_EB2C + 1], eT[:],
                             start=True, stop=True)
            rrow = cpool.tile([1, BPC], F32, tag="rrow")
            nc.vector.reciprocal(rrow[:], srow)
            rBt = cpool.tile([128, BPC], F32, tag="rBt")
            nc.gpsimd.partition_broadcast(rBt[:], rrow[:])

            def sc(b, k):
                return eB[:, b * K + k:b * K + k + 1]

            # agg bias column-path matmuls
            pag0 = rpt[:, 128:128 + BPC]
            nc.tensor.matmul(pag0, pa2[0:K, P2_BK:P2_BK + 128], eT[:],
                             start=True, stop=True)
            pag1 = rpt[:, 256:256 + BPC]
            nc.tensor.matmul(pag1, pa2[0:K, P2_BK + 128:P2_BK + 256], eT[:],
                             start=True, stop=True)

            aggb = cpool.tile([128, N_OT * BPC], F32, tag="aggb")

            # ---- mix chains ----
            # pre-allocate result tiles in sample order (bufs=4: no recycling)
            mh = {}
            for b in range(BPC):
                for it in range(N_IT):
                    mh[(b, it)] = mx16.tile([128, 256], F16, tag=f"m16_{it}",
                                            name=f"m16_{b}_{it}")

            def mix_chain(eng, b, it, lo, hi):
                m32 = mx32.tile([128, 256], F32, tag=f"m32_{it}",
                                name=f"m32_{b}_{it}_{lo}")
                m16 = mh[(b, it)]
                eng.tensor_scalar_mul(m32[:, lo:hi], wt_sb(0, it)[:, lo:hi],
                                      sc(b, 0))
                for k in range(1, K - 1):
                    eng.scalar_tensor_tensor(
                        m32[:, lo:hi], wt_sb(k, it)[:, lo:hi], sc(b, k),
                        m32[:, lo:hi], op0=ALU.mult, op1=ALU.add)
                eng.scalar_tensor_tensor(
                    m16[:, lo:hi], wt_sb(K - 1, it)[:, lo:hi], sc(b, K - 1),
                    m32[:, lo:hi], op0=ALU.mult, op1=ALU.add)

            # aggb normalization first: evicts depend on it, and the DVE
            # list-scheduler won't hoist it past the chains below
            nc.vector.tensor_tensor(aggb[:, 0:BPC], pag0, rBt[:],
                                    op=ALU.mult)
            nc.vector.tensor_tensor(aggb[:, BPC:2 * BPC], pag1, rBt[:],
                                    op=ALU.mult)

            # All mix chains on DVE (GPSIMD cannot run TensorScalarPtr on
            # real TRN2); b0's chains are ot-split for an earlier PE start.
            mix_chain(nc.vector, 0, 0, 0, 128)
            mix_chain(nc.vector, 0, 1, 0, 128)
            mix_chain(nc.vector, 0, 0, 128, 256)
            mix_chain(nc.vector, 0, 1, 128, 256)
            mix_chain(nc.vector, 1, 0, 0, 256)
            mix_chain(nc.vector, 1, 1, 0, 256)
            mix_chain(nc.vector, 2, 0, 0, 256)
            mix_chain(nc.vector, 2, 1, 0, 256)
            mix_chain(nc.vector, 3, 0, 0, 256)
            mix_chain(nc.vector, 3, 1, 0, 256)

            # ---- x loads (SP): per sample, per it, two 2048-col halves ----
            x_t = {}
            for b in range(BPC):
                for half in range(2):
                    for it in range(N_IT):
                        t = xinp.tile([128, 2048], F16, tag=f"x{it}{half}",
                                      name=f"x_{b}_{it}_{half}")
                        x_t[(b, it, half)] = t
            for b in range(BPC):
                if b == 0:
                    # interleaved 1024-col quarters: x arrives in the order
                    # the first sample's psum chunks consume it
                    for qq in range(4):
                        for it in range(N_IT):
                            t = x_t[(b, it, qq // 2)]
                            lo = (qq % 2) * 1024
                            nc.sync.dma_start(
                                t[:, lo:lo + 1024],
                                xs[b, it * 128:(it + 1) * 128,
                                   qq * 1024:(qq + 1) * 1024])
                else:
                    for half in range(2):
                        for it in range(N_IT):
                            t = x_t[(b, it, half)]
                            nc.sync.dma_start(
                                t[:], xs[b, it * 128:(it + 1) * 128,
                                         half * 2048:(half + 1) * 2048])

            osb = {}
            for b in range(BPC):
                for ot in range(N_OT):
                    if (b, ot) == (3, 1):
                        for q in range(3):
                            osb[(b, ot, q)] = o3sbp.tile(
                                [128, 1024], F16, tag="o3",
                                name=f"o3_{b}_{ot}_{q}")
                        for sq in range(2):
                            osb[(b, ot, 3, sq)] = o3sbp.tile(
                                [128, 512], F16, tag="o3s",
                                name=f"o3s_{b}_{ot}_{sq}")
                    else:
                        for hh in range(2):
                            osb[(b, ot, hh)] = osbp.tile(
                                [128, 2048], F16, tag="o",
                                name=f"o_{b}_{ot}_{hh}")

            def emit_chunk(g, b, ot, c):
                """Matmuls + eviction for one 1024-col psum chunk."""
                ps = mps.tile([128, 1024], F32, tag=f"pm{g % 4}",
                              name=f"ps_{b}_{ot}_{c}")
                half = c // 2
                for s in range(2):
                    lo = (c % 2) * 1024 + s * 512
                    for it in range(N_IT):
                        nc.tensor.matmul(
                            ps[:, s * 512:(s + 1) * 512],
                            mh[(b, it)][:, ot * 128:(ot + 1) * 128],
                            x_t[(b, it, half)][:, lo:lo + 512],
                            start=(it == 0), stop=(it == N_IT - 1))
                bias_ap = aggb[:, ot * BPC + b:ot * BPC + b + 1]
                scale_ap = rBt[:, b:b + 1]
                if g == 31:
                    # final chunk: two 512-col halves, separate out tiles,
                    # evicted concurrently on ACT and DVE
                    nc.scalar.activation(osb[(b, ot, 3, 0)][:],
                                         ps[:, 0:512], AF.Identity,
                                         bias=bias_ap, scale=scale_ap)
                    nc.vector.tensor_scalar(osb[(b, ot, 3, 1)][:],
                                            ps[:, 512:1024],
                                            scale_ap, bias_ap,
                                            op0=ALU.mult, op1=ALU.add)
                    return
                if (b, ot) == (3, 1):
                    otile, olo = osb[(b, ot, c)], 0
                else:
                    otile, olo = osb[(b, ot, c // 2)], (c % 2) * 1024
                dst = otile[:, olo:olo + 1024]
                if ev_on_dve(g):
                    nc.vector.tensor_scalar(dst, ps[:], scale_ap, bias_ap,
                                            op0=ALU.mult, op1=ALU.add)
                else:
                    nc.scalar.activation(dst, ps[:], AF.Identity,
                                         bias=bias_ap, scale=scale_ap)

            def emit_writes(b, ot):
                # ACT: early halves (slack between its evicts); Pool: mid
                # halves deferred past its mix chains; SP: all of b3 (its x
                # stream is done by then).
                if (b, ot) == (3, 1):
                    for q in range(3):
                        orows = out[b, ot * 128:(ot + 1) * 128,
                                    q * 1024:(q + 1) * 1024]
                        nc.sync.dma_start(orows, osb[(b, ot, q)][:])
                    for sq in range(2):
                        orows = out[b, ot * 128:(ot + 1) * 128,
                                    3072 + sq * 512:3072 + (sq + 1) * 512]
                        nc.sync.dma_start(orows, osb[(b, ot, 3, sq)][:])
                    return
                for hh in range(2):
                    orows = out[b, ot * 128:(ot + 1) * 128,
                                hh * 2048:(hh + 1) * 2048]
                    src = osb[(b, ot, hh)][:]
                    nc.gpsimd.dma_start(orows, src)

            # chunk order per sample: h0 chunks of both ots first, then h1
            # (x's second half is needed 3.4us into the sample, giving the
            # serial SP x-stream slack to stay ahead of PE)
            g = 0
            B0_ORDER = ((0, 0), (1, 0), (0, 1), (1, 1),
                        (0, 2), (1, 2), (0, 3), (1, 3))
            ORDER = ((0, 0), (0, 1), (1, 0), (1, 1),
                     (0, 2), (0, 3), (1, 2), (1, 3))
            for b in range(BPC):
                for ot, c in (B0_ORDER if b == 0 else ORDER):
                    emit_chunk(g, b, ot, c)
                    g += 1
                emit_writes(b, 0)
                emit_writes(b, 1)

    nc.compile()
    return nc


def _get_program():
    global _PROG
    if _PROG is None:
        _PROG = _build_program()
    return _PROG


def build_in_maps(inputs):
    x = np.asarray(inputs["x"], dtype=np.float32)
    emb = np.asarray(inputs["embedding"], dtype=np.float32)
    W0 = np.asarray(inputs["W0"], dtype=np.float32)
    b0 = np.asarray(inputs["b0"], dtype=np.float32)
    W1 = np.asarray(inputs["W1"], dtype=np.float32)
    b1 = np.asarray(inputs["b1"], dtype=np.float32)
    W2 = np.asarray(inputs["W2"], dtype=np.float32)
    b2 = np.asarray(inputs["b2"], dtype=np.float32)
    weight = np.asarray(inputs["weight"], dtype=np.float32)[..., 0]  # [K,O,I]
    bias_k = np.asarray(inputs["bias_k"], dtype=np.float32)

    x16 = x.astype(np.float16)

    expb2 = np.exp(b2.astype(np.float64))
    wscaled = weight * expb2[:, None, None].astype(np.float32)
    wta = np.ascontiguousarray(
        wscaled.transpose(2, 0, 1)            # [I, K, O]
        .reshape(N_IT, 128, K, OUT_C)         # [it, il, K, O]
        .transpose(1, 0, 2, 3)                # [il, it, K, O]
        .reshape(128, N_IT * K * OUT_C)).astype(np.float16)

    pa1 = np.zeros((128, P1A_COLS), dtype=np.float32)
    pa1[:, P1_W0:P1_W0 + HID] = W0
    pa1[:, P1_B0] = b0
    pa1b = np.zeros((128, P1B_COLS), dtype=np.float32)
    pa1b[:, P1B_W1:P1B_W1 + HID] = W1
    pa1b[:, P1B_B1] = b1
    pa1b[:, P1B_ONES] = 1.0

    pa2 = np.zeros((128, P2_COLS), dtype=np.float32)
    pa2[:, P2_W2:P2_W2 + K] = W2
    pa2[0:K, P2_EB2C] = expb2.astype(np.float32)
    pa2[0:K, P2_BK:P2_BK + OUT_C] = (
        bias_k * expb2[:, None]).astype(np.float32)

    in_maps = []
    for c in range(N_CORES):
        sl = slice(c * BPC, (c + 1) * BPC)
        p1 = pa1.copy()
        p1[:, P1_EMBT:P1_EMBT + BPC] = emb[sl].T
        in_maps.append({
            "xs": np.ascontiguousarray(x16[sl]),
            "wta": wta,
            "params1a": p1,
            "params1b": pa1b,
            "params2": pa2,
        })
    return in_maps


def run(inputs, trace=False):
    nc = _get_program()
    in_maps = build_in_maps(inputs)
    res = bass_utils.run_bass_kernel_spmd(
        nc, in_maps, core_ids=list(range(N_CORES)), trace=trace)
    out = np.concatenate(
        [res.results[c]["out"] for c in range(N_CORES)], axis=0
    ).astype(np.float32)
    return out, res


def kernel(**inputs):
    out, _ = run(inputs, trace=False)
    return out
